# revision 25
# baseline (speedup 1.0000x reference)
"""Multi-head causal attention (B=4, T=2048, D=1024, H=16) on 8 Trainium2 cores.

Sharding: core c = (b, g) with b = c//2 (batch), g = c%2 (head-group of 8 heads).
Each core computes Q/K/V projections for its 8 heads (column-parallel), causal
attention in the S^T layout (keys on partitions, queries on the free dim), and
a row-parallel partial output projection. Host sums the g=0/g=1 partials and
adds the bias.

v3 engine plan (cost-model driven):
  - Q/K projections run as fp8e4 DoubleRow matmuls (x and W pre-quantized on
    the host; W scaled by 64 so its values leave the e4m3 subnormal range, the
    scale folded back into the PSUM->SBUF copy). DoubleRow contracts 256 dims
    per instruction at 0.5 cycles/col -> 4x the fp32r rate. Scores tolerate
    the fp8 noise (it perturbs softmax weights, which average out); the V path
    does NOT (peaked rows pass quantization error straight through), so the
    V projection and everything downstream stay bf16.
  - All other matmuls are bf16 (1 cycle/col, and N<256 boundary chunks run at
    full rate, unlike fp32r): V projection, S^T = K^T Q, P^T V, out-proj.
  - exp on the Activation engine is one long pole (~123us); PE (~185us) is the
    other. Everything else hides under them.
  - Softmax denominators come free from an appended ones-column on V (even
    heads [V|1] -> Z on psum row 64; odd heads [1|0*63|V] -> Z on row 0 for
    partition_broadcast, ctx lane-aligned with ctxT[64:128]).
  - Causal handling: chunk kj only computes q-columns >= sl0 = m*128; the
    128-wide diagonal block is zeroed post-exp by a DVE multiply with a bf16
    0/1 triangular mask (bf16 everywhere -> DVE 2x_1p fast path).
  - One shared [128,512] PSUM pool serves projection accumulators, AV
    accumulators and out-proj tiles, so projections of later spans pipeline
    under the attention of earlier spans with no phase barrier.
"""

import sys

try:
    import concourse.bass  # noqa: F401
except ImportError:  # pragma: no cover
    sys.path.insert(0, "/opt/trn_rl_repo")

import numpy as np

B, T, D = 4, 2048, 1024
H, HD = 16, 64
NCORES = 8
NH = 8          # heads per core
NPAIR = 4       # head pairs per core
NSPAN = 4       # q spans of 512
SPAN = 512
NKC = 16        # key chunks of 128
KC = 128
NDC = 8         # D chunks of 128
P = 128
WSCALE = 64.0   # host-side Q/K weight scale to escape fp8 subnormals

_CACHE = {}
MMLABELS = []  # build-order labels of every PE matmul, for trace alignment


def _build():
    import concourse.bacc as bacc
    import concourse.mybir as mybir
    import concourse.tile as tile

    f32 = mybir.dt.float32
    bf16 = mybir.dt.bfloat16
    f8 = mybir.dt.float8e4
    Exp = mybir.ActivationFunctionType.Exp
    DR = mybir.MatmulPerfMode.DoubleRow

    nc = bacc.Bacc("TRN2", target_bir_lowering=False, debug=False,
                   num_devices=NCORES)

    xT_h = nc.dram_tensor("xT", (D, T), f8, kind="ExternalInput")
    xTb_h = nc.dram_tensor("xTb", (D, T), bf16, kind="ExternalInput")
    wqT_h = nc.dram_tensor("wqT", (D, 512), f8, kind="ExternalInput")
    wkT_h = nc.dram_tensor("wkT", (D, 512), f8, kind="ExternalInput")
    wvT_h = nc.dram_tensor("wvT", (D, 512), bf16, kind="ExternalInput")
    woT_h = nc.dram_tensor("woT", (512, D), bf16, kind="ExternalInput")
    out_h = nc.dram_tensor("out", (T, D), f32, kind="ExternalOutput")

    xT_d = xT_h.ap().rearrange("(dc p) t -> p dc t", p=P)       # (128, 8, 2048)
    xTb_d = xTb_h.ap().rearrange("(dc p) t -> p dc t", p=P)
    wq_d = wqT_h.ap().rearrange("(dc p) f -> p dc f", p=P)      # (128, 8, 512)
    wk_d = wkT_h.ap().rearrange("(dc p) f -> p dc f", p=P)
    wv_d = wvT_h.ap().rearrange("(dc p) f -> p dc f", p=P)
    wo_d = woT_h.ap().rearrange("(pc p) f -> p pc f", p=P)      # (128, 4, 1024)

    QKS = 1.0 / WSCALE   # W pre-scale undone here; 1/sqrt(HD) folds into exp

    MMLABELS.clear()
    _mm = nc.tensor.matmul

    def _mm_labeled(out, lhsT, rhs, label="?", **kw):
        MMLABELS.append(label)
        return _mm(out, lhsT, rhs, **kw)

    nc.tensor.matmul = _mm_labeled

    with tile.TileContext(nc) as tc:
        with (
            tc.tile_pool(name="persist", bufs=1) as persist,
            tc.tile_pool(name="ptile", bufs=6) as ppool,
            tc.tile_pool(name="xsp", bufs=2) as xpool,
            tc.tile_pool(name="xbsp", bufs=2) as xbpool,
            tc.tile_pool(name="zpool", bufs=2) as zpool,
            tc.tile_pool(name="outp", bufs=2) as outpool,
            tc.tile_pool(name="psA", bufs=4, space="PSUM") as psA,
            tc.tile_pool(name="psS", bufs=2, space="PSUM") as psS,
        ):
            # ---- persistent tiles ----
            # q/k for DoubleRow scores: [64, 2, T] fp8 per pair; partitions
            # 0:32 = head0, 32:64 = head1; the '2' free slot holds the two
            # 32-wide halves of head_dim (contraction = 2x32)
            qT = [persist.tile([HD, 2, T], f8, tag=f"qT{i}", name=f"qT{i}")
                  for i in range(NPAIR)]
            kT = [persist.tile([HD, 2, T], f8, tag=f"kT{i}", name=f"kT{i}")
                  for i in range(NPAIR)]
            ctxT = [persist.tile([P, T], bf16, tag=f"ctxT{i}", name=f"ctxT{i}")
                    for i in range(NPAIR)]
            # Even heads: [V | 1] -> AV psum rows 0:64 = ctx, row 64 = Z.
            # Odd heads: [1 | 0*63 | V] -> AV psum row 0 = Z, rows 64:128 = ctx.
            Vpe = persist.tile([P, NKC, NPAIR, HD + 1], bf16, tag="Vpe", name="Vpe")
            Vpo = persist.tile([P, NKC, NPAIR, P], bf16, tag="Vpo", name="Vpo")
            # causal diag mask as a MATMUL: ss[p, f] += Ltri[f, p] with
            # rhs = identity adds -2048 where key p > query f, so exp gives
            # exact zeros and no post-exp mask op (nor its cross-engine
            # latency) is needed. Ltri[f, p] = -2048 iff p > f; Id = I.
            Ltri = persist.tile([P, KC], bf16, tag="Ltri", name="Ltri")
            nc.gpsimd.memset(Ltri[:], -2048.0)
            nc.gpsimd.affine_select(
                out=Ltri[:], in_=Ltri[:],
                compare_op=mybir.AluOpType.is_ge, fill=0.0,
                base=-1, channel_multiplier=-1, pattern=[[1, KC]],
            )
            Idm2 = persist.tile([P, 2, KC], bf16, tag="Idm2", name="Idm2")
            nc.gpsimd.memset(Idm2[:], 1.0)
            nc.gpsimd.affine_select(
                out=Idm2[:], in_=Idm2[:],
                compare_op=mybir.AluOpType.is_equal, fill=0.0,
                base=0, channel_multiplier=-1, pattern=[[0, 2], [1, KC]],
            )

            wq = persist.tile([P, NDC, 512], f8, tag="wq", name="wq")
            wk = persist.tile([P, NDC, 512], f8, tag="wk", name="wk")
            wv = persist.tile([P, NDC, 512], bf16, tag="wv", name="wv")
            wo = persist.tile([P, 4, D], bf16, tag="wo", name="wo")
            # wq/wk interleaved on the SP queue so pr=0's q AND k projections
            # can start early; x span 0 on the scalar queue in parallel; the
            # less-urgent wv/wo go on the pool queue (25ns issue)
            nc.sync.dma_start(wq[:, 0:2], wq_d[:, 0:2])
            nc.sync.dma_start(wk[:, 0:2], wk_d[:, 0:2])
            nc.sync.dma_start(wq[:, 2:4], wq_d[:, 2:4])
            nc.sync.dma_start(wk[:, 2:4], wk_d[:, 2:4])
            nc.sync.dma_start(wq[:, 4:], wq_d[:, 4:])
            nc.sync.dma_start(wk[:, 4:], wk_d[:, 4:])
            nc.gpsimd.dma_start(wv[:], wv_d[:])
            nc.gpsimd.memset(Vpe[:, :, :, HD:HD + 1], 1.0)
            nc.gpsimd.memset(Vpo[:, :, :, 0:1], 1.0)
            nc.gpsimd.memset(Vpo[:, :, :, 1:HD], 0.0)

            # ---- projections for one span: DMA issue + PE-work closures ----
            # (closures are used as "fillers" interleaved into the attention
            # chunk loop so PE never stalls waiting on Act)
            def proj_fillers(sp):
                xt = xpool.tile([P, NDC, SPAN], f8, tag="xt", name="xt")
                xtb = xbpool.tile([P, NDC, SPAN], bf16, tag="xtb", name="xtb")
                tsl = slice(sp * SPAN, (sp + 1) * SPAN)
                nc.scalar.dma_start(xt[:, 0:2], xT_d[:, 0:2, tsl])
                nc.scalar.dma_start(xt[:, 2:], xT_d[:, 2:, tsl])
                nc.scalar.dma_start(xtb[:, 0:2], xTb_d[:, 0:2, tsl])
                nc.scalar.dma_start(xtb[:, 2:], xTb_d[:, 2:, tsl])

                def qk_one(w, dest, pr):
                    ps = psA.tile([P, SPAN], f32, tag="psA", name="psA")
                    for d2 in range(NDC // 2):
                        nc.tensor.matmul(
                            ps[:],
                            w[:, 2 * d2:2 * d2 + 2, pr * P:(pr + 1) * P],
                            xt[:, 2 * d2:2 * d2 + 2, :],
                            start=(d2 == 0), stop=(d2 == NDC // 2 - 1),
                            perf_mode=DR, label=f"qkproj s{sp} pr{pr}",
                        )
                    tsl = slice(sp * SPAN, (sp + 1) * SPAN)
                    # two DVE converts split the psum into the DoubleRow
                    # layout; the i=1 slot copy shifts -64 partitions (host W
                    # column order j = i*64 + u*32 + p)
                    nc.vector.tensor_scalar_mul(
                        dest[pr][:, 0, tsl], ps[0:HD, :], QKS)
                    nc.vector.tensor_scalar_mul(
                        dest[pr][:, 1, tsl], ps[HD:P, :], QKS)

                vps = {}

                def v_half(tb, h):
                    if h == 0:
                        vps[tb] = psA.tile([P, SPAN], f32, tag="psA", name="psA")
                    ps = vps[tb]
                    for dc in range(4 * h, 4 * h + 4):
                        nc.tensor.matmul(
                            ps[:],
                            xtb[:, dc, tb * P:(tb + 1) * P],
                            wv[:, dc],
                            start=(dc == 0), stop=(dc == NDC - 1),
                            label=f"vproj s{sp} tb{tb}",
                        )
                    if h == 1:
                        kc = sp * 4 + tb
                        psv = ps[:].rearrange(
                            "p (pr u f) -> p pr u f", u=2, f=HD)
                        nc.vector.tensor_copy(
                            Vpe[:, kc, :, 0:HD], psv[:, :, 0, :])
                        nc.vector.tensor_copy(
                            Vpo[:, kc, :, HD:P], psv[:, :, 1, :])
                        vps.pop(tb)

                fl = [(("proj", sp), lambda: qk_one(wq, qT, 0)),
                      (("proj", sp), lambda: qk_one(wk, kT, 0))]
                for tb in range(4):
                    fl.append((("proj", sp), lambda tb=tb: v_half(tb, 0)))
                    fl.append((("proj", sp), lambda tb=tb: v_half(tb, 1)))
                for pr in range(1, NPAIR):
                    fl.append((("proj", sp), lambda pr=pr: qk_one(wq, qT, pr)))
                    fl.append((("proj", sp), lambda pr=pr: qk_one(wk, kT, pr)))
                return fl

            # ---- attention for one (span, pair) ----
            def attn_pair(s, pr, fillers):
                qs = s * SPAN
                nchunk = 4 * (s + 1)
                lag = 5 if s <= 1 else 3
                ctx_ps = [psA.tile([P, SPAN], f32, tag="psA", name=f"psC{u}")
                          for u in range(2)]
                def sl0_of(kj):
                    m = kj - 4 * s
                    return 0 if m < 0 else m * KC

                pts = {}

                def av(kj):
                    sl0 = sl0_of(kj)
                    pt = pts.pop(kj)
                    nc.tensor.matmul(
                        ctx_ps[0][0:HD + 1, sl0:],
                        Vpe[:, kj, pr, :],
                        pt[:, 0, sl0:],
                        start=(kj == 0), stop=(kj == nchunk - 1),
                        label=f"av s{s} pr{pr} kj{kj}",
                    )
                    nc.tensor.matmul(
                        ctx_ps[1][0:P, sl0:],
                        Vpo[:, kj, pr, :],
                        pt[:, 1, sl0:],
                        start=(kj == 0), stop=(kj == nchunk - 1),
                        label=f"av s{s} pr{pr} kj{kj}",
                    )

                # software pipeline: AV(kj) is emitted after scores(kj+2), so
                # the in-order PE stream never blocks on exp(kj) — it always
                # has the next chunks' scores to run while Act catches up
                for kj in range(nchunk):
                    sl0 = sl0_of(kj)
                    ss = psS.tile([P, 2, SPAN], f32, tag="psS", name="psS")
                    pt = ppool.tile([P, 2, SPAN], bf16, tag="pt", name="pt")
                    pts[kj] = pt
                    bdry = kj - 4 * s >= 0
                    for u in range(2):
                        lo, hi = u * 32, (u + 1) * 32
                        nc.tensor.matmul(
                            ss[:, u, sl0:],
                            kT[pr][lo:hi, :, kj * KC:(kj + 1) * KC],
                            qT[pr][lo:hi, :, qs + sl0:qs + SPAN],
                            start=True, stop=not bdry, perf_mode=DR,
                            label=f"score s{s} pr{pr} kj{kj}",
                        )
                    if bdry:
                        # one matmul adds -2048 to both heads' causal
                        # upper-triangles: out[p,(u,f)] = Ltri[f,p]
                        nc.tensor.matmul(
                            ss[:, :, sl0:sl0 + KC], Ltri[:], Idm2[:],
                            start=False, stop=True, skip_group_check=True,
                            label=f"score s{s} pr{pr} kj{kj}",
                        )
                    nc.scalar.activation(pt[:, :, sl0:], ss[:, :, sl0:], Exp,
                                         scale=0.125)
                    if fillers and (s <= 1 or kj <= 1 or kj % 2 == 0):
                        fillers.pop(0)[1]()
                    if kj >= lag:
                        av(kj - lag)
                for kj in range(max(0, nchunk - lag), nchunk):
                    av(kj)

                # normalize + evict ctx^T (bf16)
                rz = zpool.tile([HD + 1, SPAN], f32, tag="rz", name="rz")
                nc.vector.reciprocal(rz[0:1, :], ctx_ps[0][HD:HD + 1, :])
                rzrep = zpool.tile([HD, SPAN], f32, tag="rzrep", name="rzrep")
                nc.gpsimd.partition_broadcast(rzrep[:], rz[0:1, :])
                nc.vector.tensor_mul(
                    ctxT[pr][0:HD, qs:qs + SPAN], ctx_ps[0][0:HD, :], rzrep[:])
                rzrepo = zpool.tile([P, SPAN], f32, tag="rzrepo", name="rzrepo")
                nc.vector.reciprocal(rzrepo[0:1, :], ctx_ps[1][0:1, :])
                nc.gpsimd.partition_broadcast(rzrepo[:, :], rzrepo[0:1, :])
                nc.vector.tensor_mul(
                    ctxT[pr][HD:P, qs:qs + SPAN],
                    ctx_ps[1][HD:P, :], rzrepo[HD:P, :])

            # ---- output projection (bf16): per-(tb, half) closures with a
            # direct PSUM->DRAM DMA (no SBUF staging) ----
            def outproj_one(tb):
                stage = outpool.tile([P, D], f32, tag="ostage", name="ostage")
                for os_ in range(2):
                    ps = psA.tile([P, SPAN], f32, tag="psA", name="psO")
                    for pc in range(NPAIR):
                        nc.tensor.matmul(
                            ps[:],
                            ctxT[pc][:, tb * P:(tb + 1) * P],
                            wo[:, pc, os_ * SPAN:(os_ + 1) * SPAN],
                            start=(pc == 0), stop=(pc == NPAIR - 1),
                            label=f"outproj tb{tb} os{os_}",
                        )
                    nc.vector.tensor_copy(
                        stage[:, os_ * SPAN:(os_ + 1) * SPAN], ps[:])
                nc.sync.dma_start(out_h.ap()[tb * P:(tb + 1) * P, :], stage[:])

            def outproj_fillers(s):
                return [(("op", s), lambda tb=tb: outproj_one(tb))
                        for tb in range(s * 4, (s + 1) * 4)]

            # ---- schedule: attention is the backbone; projections of span
            # s+1 and out-proj of span s-1 fill PE slack inside span s's
            # chunk loop. A span's own projections are force-drained at its
            # entry (the attention stream depends on them). ----
            fl0 = proj_fillers(0)
            for _, f in fl0[:10]:   # q0/k0 + all of V span 0
                f()
            pending = list(fl0[10:])
            nc.gpsimd.dma_start(wo[:], wo_d[:])
            for s in range(NSPAN):
                if s >= 1:
                    pending += outproj_fillers(s - 1)
                if s + 1 < NSPAN:
                    pending += proj_fillers(s + 1)
                for pr in range(NPAIR):
                    if s >= 1:
                        # run any not-yet-popped projections of THIS span
                        rest = []
                        for key, f in pending:
                            if key == ("proj", s):
                                f()
                            else:
                                rest.append((key, f))
                        pending = rest
                    attn_pair(s, pr, pending)
            for _, f in pending:
                f()
            for _, f in outproj_fillers(NSPAN - 1):
                f()

    nc.compile()
    return nc


def get_nc():
    if "nc" not in _CACHE:
        _CACHE["nc"] = _build()
    return _CACHE["nc"]


def kernel(x, Wq, Wk, Wv, Wo, bo):
    import ml_dtypes
    from concourse import bass_utils

    f8 = ml_dtypes.float8_e4m3
    bf16 = ml_dtypes.bfloat16

    x = np.asarray(x, dtype=np.float32)
    Wq, Wk, Wv = (np.asarray(w, dtype=np.float32) for w in (Wq, Wk, Wv))
    Wo = np.asarray(Wo, dtype=np.float32)
    bo = np.asarray(bo, dtype=np.float32)

    # q/k W column order (per 128-col pair): new j = half*64 + head*32 + p
    # so the projection PSUM partitions match the DoubleRow q/k layout
    perm = np.empty(512, dtype=np.int64)
    for pr in range(NPAIR):
        for i in range(2):
            for u in range(2):
                for p in range(32):
                    perm[pr * 128 + i * 64 + u * 32 + p] = \
                        pr * 128 + u * 64 + i * 32 + p

    in_maps = []
    for c in range(NCORES):
        b, g = c // 2, c % 2
        gsl = slice(g * 512, (g + 1) * 512)
        xTc = np.ascontiguousarray(x[b].T)
        in_maps.append({
            "xT": xTc.astype(f8),
            "xTb": xTc.astype(bf16),
            "wqT": np.ascontiguousarray((Wq[gsl] * WSCALE)[perm].T).astype(f8),
            "wkT": np.ascontiguousarray((Wk[gsl] * WSCALE)[perm].T).astype(f8),
            "wvT": np.ascontiguousarray(Wv[gsl].T).astype(bf16),
            "woT": np.ascontiguousarray(Wo[:, gsl].T).astype(bf16),
        })

    nc = get_nc()
    res = bass_utils.run_bass_kernel_spmd(nc, in_maps, core_ids=list(range(NCORES)))
    parts = [res.results[c]["out"] for c in range(NCORES)]
    out = np.stack([parts[2 * b] + parts[2 * b + 1] + bo for b in range(B)])
    return out.astype(np.float32)


# revision 26
# speedup vs baseline: 1.0050x; 1.0050x over previous
"""Multi-head causal attention (B=4, T=2048, D=1024, H=16) on 8 Trainium2 cores.

Sharding: core c = (b, g) with b = c//2 (batch), g = c%2 (head-group of 8 heads).
Each core computes Q/K/V projections for its 8 heads (column-parallel), causal
attention in the S^T layout (keys on partitions, queries on the free dim), and
a row-parallel partial output projection. Host sums the g=0/g=1 partials and
adds the bias.

v3 engine plan (cost-model driven):
  - Q/K projections run as fp8e4 DoubleRow matmuls (x and W pre-quantized on
    the host; W scaled by 64 so its values leave the e4m3 subnormal range, the
    scale folded back into the PSUM->SBUF copy). DoubleRow contracts 256 dims
    per instruction at 0.5 cycles/col -> 4x the fp32r rate. Scores tolerate
    the fp8 noise (it perturbs softmax weights, which average out); the V path
    does NOT (peaked rows pass quantization error straight through), so the
    V projection and everything downstream stay bf16.
  - All other matmuls are bf16 (1 cycle/col, and N<256 boundary chunks run at
    full rate, unlike fp32r): V projection, S^T = K^T Q, P^T V, out-proj.
  - exp on the Activation engine is one long pole (~123us); PE (~185us) is the
    other. Everything else hides under them.
  - Softmax denominators come free from an appended ones-column on V (even
    heads [V|1] -> Z on psum row 64; odd heads [1|0*63|V] -> Z on row 0 for
    partition_broadcast, ctx lane-aligned with ctxT[64:128]).
  - Causal handling: chunk kj only computes q-columns >= sl0 = m*128; the
    128-wide diagonal block is zeroed post-exp by a DVE multiply with a bf16
    0/1 triangular mask (bf16 everywhere -> DVE 2x_1p fast path).
  - One shared [128,512] PSUM pool serves projection accumulators, AV
    accumulators and out-proj tiles, so projections of later spans pipeline
    under the attention of earlier spans with no phase barrier.
"""

import sys

try:
    import concourse.bass  # noqa: F401
except ImportError:  # pragma: no cover
    sys.path.insert(0, "/opt/trn_rl_repo")

import numpy as np

B, T, D = 4, 2048, 1024
H, HD = 16, 64
NCORES = 8
NH = 8          # heads per core
NPAIR = 4       # head pairs per core
NSPAN = 4       # q spans of 512
SPAN = 512
NKC = 16        # key chunks of 128
KC = 128
NDC = 8         # D chunks of 128
P = 128
WSCALE = 64.0   # host-side Q/K weight scale to escape fp8 subnormals

_CACHE = {}
MMLABELS = []  # build-order labels of every PE matmul, for trace alignment


def _build():
    import concourse.bacc as bacc
    import concourse.mybir as mybir
    import concourse.tile as tile

    f32 = mybir.dt.float32
    bf16 = mybir.dt.bfloat16
    f8 = mybir.dt.float8e4
    Exp = mybir.ActivationFunctionType.Exp
    DR = mybir.MatmulPerfMode.DoubleRow

    nc = bacc.Bacc("TRN2", target_bir_lowering=False, debug=False,
                   num_devices=NCORES)

    xT_h = nc.dram_tensor("xT", (D, T), f8, kind="ExternalInput")
    xTb_h = nc.dram_tensor("xTb", (D, T), bf16, kind="ExternalInput")
    wqT_h = nc.dram_tensor("wqT", (D, 512), f8, kind="ExternalInput")
    wkT_h = nc.dram_tensor("wkT", (D, 512), f8, kind="ExternalInput")
    wvT_h = nc.dram_tensor("wvT", (D, 512), bf16, kind="ExternalInput")
    woT_h = nc.dram_tensor("woT", (512, D), bf16, kind="ExternalInput")
    out_h = nc.dram_tensor("out", (T, D), f32, kind="ExternalOutput")

    xT_d = xT_h.ap().rearrange("(dc p) t -> p dc t", p=P)       # (128, 8, 2048)
    xTb_d = xTb_h.ap().rearrange("(dc p) t -> p dc t", p=P)
    wq_d = wqT_h.ap().rearrange("(dc p) f -> p dc f", p=P)      # (128, 8, 512)
    wk_d = wkT_h.ap().rearrange("(dc p) f -> p dc f", p=P)
    wv_d = wvT_h.ap().rearrange("(dc p) f -> p dc f", p=P)
    wo_d = woT_h.ap().rearrange("(pc p) f -> p pc f", p=P)      # (128, 4, 1024)

    QKS = 1.0 / WSCALE   # W pre-scale undone here; 1/sqrt(HD) folds into exp

    MMLABELS.clear()
    _mm = nc.tensor.matmul

    def _mm_labeled(out, lhsT, rhs, label="?", **kw):
        MMLABELS.append(label)
        return _mm(out, lhsT, rhs, **kw)

    nc.tensor.matmul = _mm_labeled

    with tile.TileContext(nc) as tc:
        with (
            tc.tile_pool(name="persist", bufs=1) as persist,
            tc.tile_pool(name="ptile", bufs=10) as ppool,
            tc.tile_pool(name="xsp", bufs=2) as xpool,
            tc.tile_pool(name="xbsp", bufs=2) as xbpool,
            tc.tile_pool(name="zpool", bufs=2) as zpool,
            tc.tile_pool(name="outp", bufs=2) as outpool,
            tc.tile_pool(name="psA", bufs=4, space="PSUM") as psA,
            tc.tile_pool(name="psS", bufs=2, space="PSUM") as psS,
        ):
            # ---- persistent tiles ----
            # q/k for DoubleRow scores: [64, 2, T] fp8 per pair; partitions
            # 0:32 = head0, 32:64 = head1; the '2' free slot holds the two
            # 32-wide halves of head_dim (contraction = 2x32)
            qT = [persist.tile([HD, 2, T], f8, tag=f"qT{i}", name=f"qT{i}")
                  for i in range(NPAIR)]
            kT = [persist.tile([HD, 2, T], f8, tag=f"kT{i}", name=f"kT{i}")
                  for i in range(NPAIR)]
            ctxT = [persist.tile([P, T], bf16, tag=f"ctxT{i}", name=f"ctxT{i}")
                    for i in range(NPAIR)]
            # Even heads: [V | 1] -> AV psum rows 0:64 = ctx, row 64 = Z.
            # Odd heads: [1 | 0*63 | V] -> AV psum row 0 = Z, rows 64:128 = ctx.
            Vpe = persist.tile([P, NKC, NPAIR, HD + 1], bf16, tag="Vpe", name="Vpe")
            Vpo = persist.tile([P, NKC, NPAIR, P], bf16, tag="Vpo", name="Vpo")
            # causal diag mask as a MATMUL: ss[p, f] += Ltri[f, p] with
            # rhs = identity adds -2048 where key p > query f, so exp gives
            # exact zeros and no post-exp mask op (nor its cross-engine
            # latency) is needed. Ltri[f, p] = -2048 iff p > f; Id = I.
            Ltri = persist.tile([P, KC], bf16, tag="Ltri", name="Ltri")
            nc.gpsimd.memset(Ltri[:], -2048.0)
            nc.gpsimd.affine_select(
                out=Ltri[:], in_=Ltri[:],
                compare_op=mybir.AluOpType.is_ge, fill=0.0,
                base=-1, channel_multiplier=-1, pattern=[[1, KC]],
            )
            Idm2 = persist.tile([P, 2, KC], bf16, tag="Idm2", name="Idm2")
            nc.gpsimd.memset(Idm2[:], 1.0)
            nc.gpsimd.affine_select(
                out=Idm2[:], in_=Idm2[:],
                compare_op=mybir.AluOpType.is_equal, fill=0.0,
                base=0, channel_multiplier=-1, pattern=[[0, 2], [1, KC]],
            )

            wq = persist.tile([P, NDC, 512], f8, tag="wq", name="wq")
            wk = persist.tile([P, NDC, 512], f8, tag="wk", name="wk")
            wv = persist.tile([P, NDC, 512], bf16, tag="wv", name="wv")
            wo = persist.tile([P, 4, D], bf16, tag="wo", name="wo")
            # wq/wk interleaved on the SP queue so pr=0's q AND k projections
            # can start early; x span 0 on the scalar queue in parallel; the
            # less-urgent wv/wo go on the pool queue (25ns issue)
            nc.sync.dma_start(wq[:, 0:2], wq_d[:, 0:2])
            nc.sync.dma_start(wk[:, 0:2], wk_d[:, 0:2])
            nc.sync.dma_start(wq[:, 2:], wq_d[:, 2:])
            nc.sync.dma_start(wk[:, 2:], wk_d[:, 2:])
            nc.gpsimd.dma_start(wv[:], wv_d[:])
            nc.gpsimd.memset(Vpe[:, :, :, HD:HD + 1], 1.0)
            nc.gpsimd.memset(Vpo[:, :, :, 0:1], 1.0)
            nc.gpsimd.memset(Vpo[:, :, :, 1:HD], 0.0)

            # ---- projections for one span: DMA issue + PE-work closures ----
            # (closures are used as "fillers" interleaved into the attention
            # chunk loop so PE never stalls waiting on Act)
            def proj_fillers(sp):
                xt = xpool.tile([P, NDC, SPAN], f8, tag="xt", name="xt")
                xtb = xbpool.tile([P, NDC, SPAN], bf16, tag="xtb", name="xtb")
                tsl = slice(sp * SPAN, (sp + 1) * SPAN)
                nc.scalar.dma_start(xt[:, 0:2], xT_d[:, 0:2, tsl])
                nc.scalar.dma_start(xt[:, 2:], xT_d[:, 2:, tsl])
                nc.scalar.dma_start(xtb[:, 0:2], xTb_d[:, 0:2, tsl])
                nc.scalar.dma_start(xtb[:, 2:], xTb_d[:, 2:, tsl])

                def qk_one(w, dest, pr):
                    ps = psA.tile([P, SPAN], f32, tag="psA", name="psA")
                    for d2 in range(NDC // 2):
                        nc.tensor.matmul(
                            ps[:],
                            w[:, 2 * d2:2 * d2 + 2, pr * P:(pr + 1) * P],
                            xt[:, 2 * d2:2 * d2 + 2, :],
                            start=(d2 == 0), stop=(d2 == NDC // 2 - 1),
                            perf_mode=DR, label=f"qkproj s{sp} pr{pr}",
                        )
                    tsl = slice(sp * SPAN, (sp + 1) * SPAN)
                    # two DVE converts split the psum into the DoubleRow
                    # layout; the i=1 slot copy shifts -64 partitions (host W
                    # column order j = i*64 + u*32 + p)
                    nc.vector.tensor_scalar_mul(
                        dest[pr][:, 0, tsl], ps[0:HD, :], QKS)
                    nc.vector.tensor_scalar_mul(
                        dest[pr][:, 1, tsl], ps[HD:P, :], QKS)

                vps = {}

                def v_half(tb, h):
                    if h == 0:
                        vps[tb] = psA.tile([P, SPAN], f32, tag="psA", name="psA")
                    ps = vps[tb]
                    for dc in range(4 * h, 4 * h + 4):
                        nc.tensor.matmul(
                            ps[:],
                            xtb[:, dc, tb * P:(tb + 1) * P],
                            wv[:, dc],
                            start=(dc == 0), stop=(dc == NDC - 1),
                            label=f"vproj s{sp} tb{tb}",
                        )
                    if h == 1:
                        kc = sp * 4 + tb
                        psv = ps[:].rearrange(
                            "p (pr u f) -> p pr u f", u=2, f=HD)
                        nc.vector.tensor_copy(
                            Vpe[:, kc, :, 0:HD], psv[:, :, 0, :])
                        nc.vector.tensor_copy(
                            Vpo[:, kc, :, HD:P], psv[:, :, 1, :])
                        vps.pop(tb)

                fl = [(("proj", sp), lambda: qk_one(wq, qT, 0)),
                      (("proj", sp), lambda: qk_one(wk, kT, 0))]
                for tb in range(4):
                    fl.append((("proj", sp), lambda tb=tb: v_half(tb, 0)))
                    fl.append((("proj", sp), lambda tb=tb: v_half(tb, 1)))
                for pr in range(1, NPAIR):
                    fl.append((("proj", sp), lambda pr=pr: qk_one(wq, qT, pr)))
                    fl.append((("proj", sp), lambda pr=pr: qk_one(wk, kT, pr)))
                return fl

            # ---- attention for one (span, pair) ----
            def attn_pair(s, pr, fillers):
                qs = s * SPAN
                nchunk = 4 * (s + 1)
                lag = nchunk if s <= 1 else 3
                ctx_ps = [psA.tile([P, SPAN], f32, tag="psA", name=f"psC{u}")
                          for u in range(2)]
                def sl0_of(kj):
                    m = kj - 4 * s
                    return 0 if m < 0 else m * KC

                pts = {}

                def av(kj):
                    sl0 = sl0_of(kj)
                    pt = pts.pop(kj)
                    nc.tensor.matmul(
                        ctx_ps[0][0:HD + 1, sl0:],
                        Vpe[:, kj, pr, :],
                        pt[:, 0, sl0:],
                        start=(kj == 0), stop=(kj == nchunk - 1),
                        label=f"av s{s} pr{pr} kj{kj}",
                    )
                    nc.tensor.matmul(
                        ctx_ps[1][0:P, sl0:],
                        Vpo[:, kj, pr, :],
                        pt[:, 1, sl0:],
                        start=(kj == 0), stop=(kj == nchunk - 1),
                        label=f"av s{s} pr{pr} kj{kj}",
                    )

                # software pipeline: AV(kj) is emitted after scores(kj+2), so
                # the in-order PE stream never blocks on exp(kj) — it always
                # has the next chunks' scores to run while Act catches up
                for kj in range(nchunk):
                    sl0 = sl0_of(kj)
                    ss = psS.tile([P, 2, SPAN], f32, tag="psS", name="psS")
                    pt = ppool.tile([P, 2, SPAN], bf16, tag="pt", name="pt")
                    pts[kj] = pt
                    bdry = kj - 4 * s >= 0
                    for u in range(2):
                        lo, hi = u * 32, (u + 1) * 32
                        nc.tensor.matmul(
                            ss[:, u, sl0:],
                            kT[pr][lo:hi, :, kj * KC:(kj + 1) * KC],
                            qT[pr][lo:hi, :, qs + sl0:qs + SPAN],
                            start=True, stop=not bdry, perf_mode=DR,
                            label=f"score s{s} pr{pr} kj{kj}",
                        )
                    if bdry:
                        # one matmul adds -2048 to both heads' causal
                        # upper-triangles: out[p,(u,f)] = Ltri[f,p]
                        nc.tensor.matmul(
                            ss[:, :, sl0:sl0 + KC], Ltri[:], Idm2[:],
                            start=False, stop=True, skip_group_check=True,
                            label=f"score s{s} pr{pr} kj{kj}",
                        )
                    nc.scalar.activation(pt[:, :, sl0:], ss[:, :, sl0:], Exp,
                                         scale=0.125)
                    if fillers and (s <= 1 or kj <= 1 or kj % 2 == 0):
                        fillers.pop(0)[1]()
                    if kj >= lag:
                        av(kj - lag)
                for kj in range(max(0, nchunk - lag), nchunk):
                    av(kj)

                # normalize + evict ctx^T (bf16)
                rz = zpool.tile([HD + 1, SPAN], f32, tag="rz", name="rz")
                nc.vector.reciprocal(rz[0:1, :], ctx_ps[0][HD:HD + 1, :])
                rzrep = zpool.tile([HD, SPAN], f32, tag="rzrep", name="rzrep")
                nc.gpsimd.partition_broadcast(rzrep[:], rz[0:1, :])
                nc.vector.tensor_mul(
                    ctxT[pr][0:HD, qs:qs + SPAN], ctx_ps[0][0:HD, :], rzrep[:])
                rzrepo = zpool.tile([P, SPAN], f32, tag="rzrepo", name="rzrepo")
                nc.vector.reciprocal(rzrepo[0:1, :], ctx_ps[1][0:1, :])
                nc.gpsimd.partition_broadcast(rzrepo[:, :], rzrepo[0:1, :])
                nc.vector.tensor_mul(
                    ctxT[pr][HD:P, qs:qs + SPAN],
                    ctx_ps[1][HD:P, :], rzrepo[HD:P, :])

            # ---- output projection (bf16): per-(tb, half) closures with a
            # direct PSUM->DRAM DMA (no SBUF staging) ----
            def outproj_one(tb):
                stage = outpool.tile([P, D], f32, tag="ostage", name="ostage")
                for os_ in range(2):
                    ps = psA.tile([P, SPAN], f32, tag="psA", name="psO")
                    for pc in range(NPAIR):
                        nc.tensor.matmul(
                            ps[:],
                            ctxT[pc][:, tb * P:(tb + 1) * P],
                            wo[:, pc, os_ * SPAN:(os_ + 1) * SPAN],
                            start=(pc == 0), stop=(pc == NPAIR - 1),
                            label=f"outproj tb{tb} os{os_}",
                        )
                    nc.vector.tensor_copy(
                        stage[:, os_ * SPAN:(os_ + 1) * SPAN], ps[:])
                nc.sync.dma_start(out_h.ap()[tb * P:(tb + 1) * P, :], stage[:])

            def outproj_fillers(s):
                return [(("op", s), lambda tb=tb: outproj_one(tb))
                        for tb in range(s * 4, (s + 1) * 4)]

            # ---- schedule: attention is the backbone; projections of span
            # s+1 and out-proj of span s-1 fill PE slack inside span s's
            # chunk loop. A span's own projections are force-drained at its
            # entry (the attention stream depends on them). ----
            fl0 = proj_fillers(0)
            for _, f in fl0[:10]:   # q0/k0 + all of V span 0
                f()
            pending = list(fl0[10:])
            nc.gpsimd.dma_start(wo[:], wo_d[:])
            for s in range(NSPAN):
                if s >= 1:
                    pending += outproj_fillers(s - 1)
                if s + 1 < NSPAN:
                    pending += proj_fillers(s + 1)
                for pr in range(NPAIR):
                    if s >= 1:
                        # run any not-yet-popped projections of THIS span
                        rest = []
                        for key, f in pending:
                            if key == ("proj", s):
                                f()
                            else:
                                rest.append((key, f))
                        pending = rest
                    attn_pair(s, pr, pending)
            for _, f in pending:
                f()
            for _, f in outproj_fillers(NSPAN - 1):
                f()

    nc.compile()
    return nc


def get_nc():
    if "nc" not in _CACHE:
        _CACHE["nc"] = _build()
    return _CACHE["nc"]


def kernel(x, Wq, Wk, Wv, Wo, bo):
    import ml_dtypes
    from concourse import bass_utils

    f8 = ml_dtypes.float8_e4m3
    bf16 = ml_dtypes.bfloat16

    x = np.asarray(x, dtype=np.float32)
    Wq, Wk, Wv = (np.asarray(w, dtype=np.float32) for w in (Wq, Wk, Wv))
    Wo = np.asarray(Wo, dtype=np.float32)
    bo = np.asarray(bo, dtype=np.float32)

    # q/k W column order (per 128-col pair): new j = half*64 + head*32 + p
    # so the projection PSUM partitions match the DoubleRow q/k layout
    perm = np.empty(512, dtype=np.int64)
    for pr in range(NPAIR):
        for i in range(2):
            for u in range(2):
                for p in range(32):
                    perm[pr * 128 + i * 64 + u * 32 + p] = \
                        pr * 128 + u * 64 + i * 32 + p

    in_maps = []
    for c in range(NCORES):
        b, g = c // 2, c % 2
        gsl = slice(g * 512, (g + 1) * 512)
        xTc = np.ascontiguousarray(x[b].T)
        in_maps.append({
            "xT": xTc.astype(f8),
            "xTb": xTc.astype(bf16),
            "wqT": np.ascontiguousarray((Wq[gsl] * WSCALE)[perm].T).astype(f8),
            "wkT": np.ascontiguousarray((Wk[gsl] * WSCALE)[perm].T).astype(f8),
            "wvT": np.ascontiguousarray(Wv[gsl].T).astype(bf16),
            "woT": np.ascontiguousarray(Wo[:, gsl].T).astype(bf16),
        })

    nc = get_nc()
    res = bass_utils.run_bass_kernel_spmd(nc, in_maps, core_ids=list(range(NCORES)))
    parts = [res.results[c]["out"] for c in range(NCORES)]
    out = np.stack([parts[2 * b] + parts[2 * b + 1] + bo for b in range(B)])
    return out.astype(np.float32)
